# revision 48
# baseline (speedup 1.0000x reference)
"""Multi-head graph-attention (GAT) kernel for Trainium2, 8 NeuronCores.

Reference computation (per head):
    h_prime = h @ w[head]                       # [8192, 64]
    s = h_prime @ a_src[head],  d = h_prime @ a_dst[head]
    attn = softmax_j(leaky_relu(s_i + d_j, 0.2))
    out  = attn @ h_prime + bias                # -> [8192, 4*64]

Low-rank reformulation (no O(n^2) work on device):
    W[i,j] = exp(lrelu(s_i + d_j)) = e^{s_i} e^{d_j} + K(s_i, d_j)
  where K(s,d) = exp(0.2(s+d)) - exp(s+d) for s+d < 0, else 0, is a bounded
  continuous function on the (s,d) rectangle covered by the data.  K is
  approximated by a low-rank product expansion fitted ON THE HOST per head
  (ridge least squares; coefficients and ranges become runtime inputs):
    K(s,d) ~= sum_{a,b} C[a,b] f_a(s/Ls) g_b(d/Ld)
  f_a = Chebyshev T_a, a < S_RANK (serial DVE recurrence, small i side),
  g_b = tanh soft-steps {1, tanh(BETA(x - t_b))}, ND_RANK of them
  (independent, chain-free Tanh activations on the scalar engine for the
  large j side, issued in two row-halves overlapping the h' build; unused
  basis columns of the fixed 32-column layout are zeroed).
  With Haug = [h' | s^ d^ | 1] and TDaug = [g_b(d^) | e^d] (j on partitions):
    B      = TDaug^T @ Haug                      # [32, 67]   (PE, O(n R))
    BKaug4 = E^T @ B                             # [128, 67]  one matmul; E is
             host-built: rows 32k+a = sum_b C[a,b] B_gb + e^d row at 32k+31,
             replicated for k=0..3 and pre-scaled by 2^-8
    out^T  = Faug-tile^T @ BKaug4[32k:32k+32]    # [128 i, 67] per i-tile
  where Faug = [T_0..T_30 | e^s], transposed 4 i-tiles at a time (partition
  bands 0/32/64/96 + matching tile_position).  Row 66 of out^T is the softmax
  denominator; the epilogue divides (+bias) and stores.  Max rel err vs the
  reference is ~2.7e-3 (fp16 operands, fp32 psum accumulation): 7x inside
  the 2e-2 gate.  h' is computed with a host-built block-diagonal waug so
  one full-K matmul serves the two j-tiles stacked per hT2 block.

Sharding: 8 cores = 4 heads x 2 row-halves (head parallel + bs row shard).
Each core gets full h (rotated so its own half is first) plus per-head
host-fitted constants; no collectives.
"""

import numpy as np

import concourse.bass as bass
import concourse.tile as tile
from concourse import bacc, mybir
from concourse.bass_utils import run_bass_kernel_spmd
from concourse.masks import make_identity

F32 = mybir.dt.float32
F16 = mybir.dt.float16
AF = mybir.ActivationFunctionType

BS = 8192          # nodes
F = 64             # f_in == f_out
NH = 4             # heads
HALF = BS // 2     # rows per core (row-half)
NT = BS // 128     # 64 j tiles
NTI = HALF // 128  # 32 i tiles
R = 31             # expansion rank per side
MB = R + 1         # TDaug / Faug columns (basis + e^x)
MH = 67            # Haug columns: h'(64), 2 zero, ones
S_RANK = 24        # Chebyshev rank actually used on the s side
ND_RANK = 10       # number of tanh soft-steps on the d side
BETA = 5.0         # tanh soft-step sharpness (d-side basis)
SHIFT_EXT = 1.1    # shift extent of the tanh steps
ALPHA = 0.2
ND_SCALE = 2.0 ** -8   # folded into E so num/den stay small


def _build_kernel_module(has_bias):
    nc = bacc.Bacc("TRN2", target_bir_lowering=False, debug=False)

    h_d = nc.dram_tensor("hfull", [BS, F], F32, kind="ExternalInput")
    # waug_bd: block-diagonal [waug 0; 0 waug], waug = [w | ws/Ls | wd/Ld]
    waug_d = nc.dram_tensor("waug", [128, 2 * (F + 2)], F32,
                            kind="ExternalInput")
    # E: [MB, 128] expansion matrix (fit coeffs -> 4x replicated BKaug rows)
    e_d = nc.dram_tensor("emat", [MB, 128], F32, kind="ExternalInput")
    # scal: [128, 2] (Ls, Ld) replicated across partitions
    scal_d = nc.dram_tensor("scal", [128, 2], F32, kind="ExternalInput")
    if has_bias:
        biasr_d = nc.dram_tensor("biasr", [128, F], F32, kind="ExternalInput")
    out_d = nc.dram_tensor("out", [HALF, F], F32, kind="ExternalOutput")

    with tile.TileContext(nc) as tc:
        with (
            tc.tile_pool(name="const", bufs=1) as cpool,
            tc.tile_pool(name="work", bufs=3) as wpool,
            tc.tile_pool(name="psum", bufs=2, space="PSUM") as ppool,
        ):
            # first h block DMA goes on the queue BEFORE the const DMAs so
            # the PE can start transposing as early as possible
            hview = h_d.ap().rearrange("(a p) f -> p a f", p=128)
            ldb0 = wpool.tile([128, 8 * F], F32, tag="ldb", bufs=6)
            nc.sync.dma_start(ldb0[:], hview[:, 0:8, :])

            # ---------------- constants ----------------
            ident32 = cpool.tile([128, 128], F32)
            make_identity(nc, ident32[:])
            ident16 = cpool.tile([128, 128], F16)
            make_identity(nc, ident16[:])

            # block-diagonal waug: one full-K matmul emits h' for the two
            # j-tiles stacked on the partition halves of an hT2 block
            waug_sb = cpool.tile([128, 2 * (F + 2)], F32)
            nc.sync.dma_start(waug_sb[:], waug_d.ap())
            waug16 = cpool.tile([128, 2 * (F + 2)], F16)
            nc.vector.tensor_copy(waug16[:], waug_sb[:])

            e_sb = cpool.tile([MB, 128], F32)
            nc.sync.dma_start(e_sb[:], e_d.ap())
            e16 = cpool.tile([MB, 128], F16)

            scal_sb = cpool.tile([128, 2], F32)
            nc.sync.dma_start(scal_sb[:], scal_d.ap())
            if has_bias:
                biasr = cpool.tile([128, F], F32)
                nc.sync.dma_start(biasr[:], biasr_d.ap())

            # ---------------- big SBUF tensors ----------------
            # h^T in 2-tile blocks: block t holds j-tiles (2t, 2t+1) on
            # partition halves 0-63 / 64-127
            hT2 = cpool.tile([128, (NT // 2) * 128], F16)
            Haug = cpool.tile([128, NT * MH], F16)      # [h' | 0 0 | 1]
            Haug3 = Haug[:].rearrange("p (t c) -> p t c", c=MH)
            sd = cpool.tile([128, NT * 2], F32)         # [s^, d^] per j tile
            sd3 = sd[:].rearrange("p (t c) -> p t c", c=2)
            TDaug = cpool.tile([128, NT * MB], F16)     # [tanh-steps(d^) | e^d]
            TDaug3 = TDaug[:].rearrange("p (t c) -> p t c", c=MB)
            Fi = cpool.tile([128, NTI * MB], F16)       # [cheb(s^) | e^s]
            Fi3 = Fi[:].rearrange("p (t c) -> p t c", c=MB)
            S_all = cpool.tile([128, R * NTI], F32)     # cheb fp32 state
            S3 = S_all[:].rearrange("p (b t) -> p b t", t=NTI)
            # Faug^T: block q holds i-tiles 4q..4q+3 on partition bands 32k
            F3t = cpool.tile([128, (NTI // 4) * 128], F16)
            F33 = F3t[:].rearrange("p (t c) -> p t c", c=128)
            BKaug4 = cpool.tile([128, MH], F16)

            def init_memsets():
                nc.gpsimd.memset(Haug3[:, :, F : F + 2], 0.0)
                nc.gpsimd.memset(Haug3[:, :, MH - 1], 1.0)
                nc.gpsimd.memset(TDaug3[:, :, 0], 1.0)
                nc.gpsimd.memset(TDaug3[:, :, 1 + ND_RANK : MB - 1], 0.0)
                nc.gpsimd.memset(Fi3[:, :, S_RANK : MB - 1], 0.0)
                nc.gpsimd.memset(Fi3[:, :, 0], 1.0)
                nc.gpsimd.memset(S3[:, 0, :], 1.0)

            def cp(e, out, in_):
                (e.copy if e is nc.scalar else e.tensor_copy)(out, in_)

            # ---------------- phase 1 block worker ----------------
            # PSUM-reading copies only on vector/scalar (GPSIMD has no PSUM
            # access).  eng_ht drains the transposes, eng_hs drains h'.
            def phase1_block(blk, eng_ht, eng_hs, ldb=None, f32tr=False,
                             merge_sd=False):
                if ldb is None:
                    ldb = wpool.tile([128, 8 * F], F32, tag="ldb", bufs=6)
                    nc.sync.dma_start(
                        ldb[:], hview[:, blk * 8 : (blk + 1) * 8, :]
                    )
                if f32tr:
                    ldb16 = ldb
                    ident, tdt = ident32, F32
                else:
                    ldb16 = wpool.tile([128, 8 * F], F16, tag="ldb16", bufs=4)
                    nc.gpsimd.tensor_copy(ldb16[:], ldb[:])
                    ident, tdt = ident16, F16
                for g in range(2):
                    jt0 = blk * 8 + g * 4
                    trp2 = ppool.tile([128, 256], tdt, tag="tr", bufs=3)
                    for k in range(2):
                        nc.tensor.transpose(
                            trp2[:, k * 128 : (k + 1) * 128],
                            ldb16[:, (g * 4 + 2 * k) * F : (g * 4 + 2 * k + 2) * F],
                            ident[:],
                        )
                    blk2 = jt0 // 2
                    cp(eng_ht[g], hT2[:, blk2 * 128 : (blk2 + 2) * 128], trp2[:])
                    hp4 = ppool.tile([128, 4 * (F + 2)], F32, tag="hp", bufs=4)
                    for b2 in range(2):
                        nc.tensor.matmul(
                            hp4[:, b2 * 2 * (F + 2) : (b2 + 1) * 2 * (F + 2)],
                            hT2[:, (blk2 + b2) * 128 : (blk2 + b2 + 1) * 128],
                            waug16[:],
                        )
                    h3 = hp4[:].rearrange("p (t c) -> p t c", c=F + 2)
                    if merge_sd:
                        cp(eng_hs[g], Haug3[:, jt0 : jt0 + 4, 0 : F + 2],
                           h3[:, :, 0 : F + 2])
                    else:
                        cp(eng_hs[g], Haug3[:, jt0 : jt0 + 4, 0:F],
                           h3[:, :, 0:F])
                        cp(eng_hs[(g + 1) % 2], sd3[:, jt0 : jt0 + 4, :],
                           h3[:, :, F : F + 2])

            # ---------------- d side: tanh soft-step basis via scalar ACTs --
            # g_b(x) = tanh(BETA * (x - t_b)): one chain-free ACT per column
            shifts = np.linspace(-SHIFT_EXT, SHIFT_EXT, ND_RANK)
            tb_bias = cpool.tile([128, ND_RANK], F32)

            def d_tanh(lo, hi, b0=0, b1=ND_RANK, with_exp=True, from_haug=False):
                dv = Haug3[:, lo:hi, F + 1] if from_haug else sd3[:, lo:hi, 1]
                if with_exp:
                    nc.scalar.activation(
                        TDaug3[:, lo:hi, MB - 1], dv, AF.Exp,
                        scale=scal_sb[:, 1:2],
                    )
                for b in range(b0, b1):
                    nc.scalar.activation(
                        TDaug3[:, lo:hi, 1 + b], dv, AF.Tanh,
                        scale=BETA, bias=tb_bias[:, b : b + 1],
                    )

            # ---------------- s side: Chebyshev recurrence on DVE ----------
            s_view = sd3[:, 0:NTI, 0]
            x2s = cpool.tile([128, NTI], F32)

            def s_cheb_setup():
                nc.scalar.activation(
                    Fi3[:, :, MB - 1], s_view, AF.Exp, scale=scal_sb[:, 0:1]
                )
                nc.gpsimd.tensor_copy(Fi3[:, :, 1], s_view)
                nc.vector.tensor_copy(S3[:, 1, :], s_view)
                nc.vector.tensor_scalar_mul(x2s[:], s_view, 2.0)

            def s_cheb_chunk(b0, b1):
                for b in range(b0, b1):
                    tmp = wpool.tile([128, NTI], F32, tag="tmps", bufs=2)
                    nc.vector.tensor_mul(tmp[:], x2s[:], S3[:, b - 1, :])
                    nc.vector.tensor_sub(S3[:, b, :], tmp[:], S3[:, b - 2, :])
                    if b % 2 == 0:
                        nc.gpsimd.tensor_copy(
                            Fi3[:, :, b - 1 : b + 1],
                            S3[:, b - 1 : b + 1, :].rearrange("p b t -> p t b"),
                        )
                    elif b == S_RANK - 1:
                        nc.gpsimd.tensor_copy(
                            Fi3[:, :, b : b + 1],
                            S3[:, b : b + 1, :].rearrange("p b t -> p t b"),
                        )

            # ---------------- emit program ----------------
            phase1_block(0, [nc.vector, nc.scalar], [nc.scalar, nc.vector],
                         ldb=ldb0, f32tr=True)
            phase1_block(1, [nc.scalar, nc.vector], [nc.vector, nc.scalar])
            init_memsets()
            nc.gpsimd.tensor_copy(e16[:], e_sb[:])
            for b, t in enumerate(shifts):
                nc.gpsimd.memset(tb_bias[:, b : b + 1], float(-BETA * t))
            for blk in range(2, 4):
                et = [nc.vector, nc.scalar] if blk % 2 else [nc.scalar, nc.vector]
                phase1_block(blk, et, et[::-1])
            s_cheb_setup()
            d_tanh(0, 32)               # scalar, overlaps blocks 4-7
            s_chunks = [(2, 8), (8, 13), (13, 18), (18, S_RANK)]
            for blk in range(4, 8):
                phase1_block(blk, [nc.scalar, nc.vector], [nc.scalar, nc.vector],
                             merge_sd=True)
                s_cheb_chunk(*s_chunks[blk - 4])
            d_tanh(32, 64, from_haug=True)  # after block 7's merged drain

            # ---------------- B = TDaug^T @ Haug ----------------
            # split in halves so the Faug transposes can fill the PE while
            # the second-half d basis finishes
            B_ps = ppool.tile([MB, MH], F32, tag="acc", bufs=1)
            for jt in range(32):
                nc.tensor.matmul(
                    B_ps[:], TDaug3[:, jt, :], Haug3[:, jt, :],
                    start=(jt == 0), stop=False,
                )

            # transpose Faug (4 i-tiles per go)
            for q in range(NTI // 4):
                ftp = ppool.tile([128, 128], F16, tag="tr", bufs=3)
                nc.tensor.transpose(
                    ftp[:], Fi3[:, 4 * q : 4 * q + 4, :], ident16[:]
                )
                cp((nc.vector, nc.scalar)[q % 2], F33[:, q, :], ftp[:])

            for jt in range(32, NT):
                nc.tensor.matmul(
                    B_ps[:], TDaug3[:, jt, :], Haug3[:, jt, :],
                    start=False, stop=(jt == NT - 1),
                )
            B16 = cpool.tile([MB, MH], F16)
            nc.vector.tensor_copy(B16[:], B_ps[:])

            # BKaug4 = E^T @ B  (4x replicated, pre-scaled)
            bk_ps = ppool.tile([128, MH], F32, tag="tr", bufs=3)
            nc.tensor.matmul(bk_ps[:], e16[:], B16[:])
            nc.vector.tensor_copy(BKaug4[:], bk_ps[:])

            # ---------------- synthesis + epilogue ----------------
            out_view = out_d.ap().rearrange("(a p) f -> p a f", p=128)
            for ch in range(NTI // 4):
                o1c = wpool.tile([128, 4 * F], F32, tag="o1c", bufs=3)
                for sub in range(4):
                    it = ch * 4 + sub
                    q, k = it // 4, it % 4
                    lo = 32 * k
                    ot_ps = ppool.tile([128, MH], F32, tag="hp", bufs=4)
                    nc.tensor.matmul(
                        ot_ps[:],
                        F33[lo : lo + 32, q, :],
                        BKaug4[lo : lo + 32, :],
                        tile_position=(lo, 0),
                    )
                    rec = wpool.tile([128, 1], F32, tag="rec", bufs=4)
                    nc.vector.reciprocal(rec[:], ot_ps[:, MH - 1 : MH])
                    if has_bias:
                        o1 = wpool.tile([128, F], F32, tag="o1", bufs=4)[:]
                    else:
                        o1 = o1c[:, sub * F : (sub + 1) * F]
                    if sub % 2 == 0:
                        nc.scalar.mul(o1, ot_ps[:, 0:F], rec[:])
                    else:
                        nc.vector.tensor_scalar_mul(o1, ot_ps[:, 0:F], rec[:])
                    if has_bias:
                        nc.gpsimd.tensor_add(
                            o1c[:, sub * F : (sub + 1) * F], o1, biasr[:]
                        )
                nc.sync.dma_start(
                    out_view[:, ch * 4 : (ch + 1) * 4, :], o1c[:]
                )

    nc.compile()
    return nc


_NC_CACHE = {}


def _get_nc(has_bias):
    if has_bias not in _NC_CACHE:
        _NC_CACHE[has_bias] = _build_kernel_module(has_bias)
    return _NC_CACHE[has_bias]


def _step_basis(xh):
    cols = [np.ones_like(xh)]
    for t in np.linspace(-SHIFT_EXT, SHIFT_EXT, ND_RANK):
        cols.append(np.tanh(BETA * (xh - t)))
    return np.stack(cols, axis=1)


def _fit_K(Ls, Ld, ngrid=160, lam=1e-10):
    """Least-squares fit of K(s,d) = exp(.2(s+d))-exp(s+d) (s+d<0) over
    [-Ls,Ls] x [-Ld,Ld] in the product basis cheb(s) x fourier(d).
    Returns C [R, R] (s-basis x d-basis)."""
    gs = np.linspace(-1.0, 1.0, ngrid)
    S, D = np.meshgrid(gs * Ls, gs * Ld, indexing="ij")
    X = S + D
    K = np.where(X < 0, np.exp(ALPHA * X) - np.exp(X), 0.0)
    Bs = np.polynomial.chebyshev.chebvander(gs, S_RANK - 1)
    Bd = _step_basis(gs)

    def pinv(B):
        U, sv, Vt = np.linalg.svd(B, full_matrices=False)
        return (Vt.T * (sv / (sv ** 2 + lam))) @ U.T

    return pinv(Bs) @ K @ pinv(Bd).T


def _make_in_maps(h, w, a_src, a_dst, bias):
    h = np.ascontiguousarray(np.asarray(h, dtype=np.float32))
    w = np.asarray(w, dtype=np.float32)
    a_src = np.asarray(a_src, dtype=np.float32)
    a_dst = np.asarray(a_dst, dtype=np.float32)
    bias = np.asarray(bias, dtype=np.float32)
    has_bias = bool(np.any(bias != 0.0))
    biasr = np.ascontiguousarray(np.broadcast_to(bias[None, :], (128, F)))

    in_maps = []
    for c in range(8):
        head, half = c // 2, c % 2
        ws = w[head] @ a_src[head][:, 0]          # [64]
        wd = w[head] @ a_dst[head][:, 0]
        s_all = h @ ws
        d_all = h @ wd
        Ls = float(np.abs(s_all).max()) * 1.02 + 1e-30
        Ld = float(np.abs(d_all).max()) * 1.02 + 1e-30
        C = _fit_K(Ls, Ld)
        waug = np.concatenate(
            [w[head], (ws / Ls)[:, None], (wd / Ld)[:, None]], axis=1
        ).astype(np.float32)
        wbd = np.zeros((128, 2 * (F + 2)), dtype=np.float32)
        wbd[0:F, 0 : F + 2] = waug
        wbd[F:128, F + 2 : 2 * (F + 2)] = waug
        # E[b, 32k+a] = scale*C[a, b]; unused rows/cols stay 0; e^d at row R
        E = np.zeros((MB, 128), dtype=np.float32)
        for k in range(4):
            E[0 : 1 + ND_RANK, 32 * k : 32 * k + S_RANK] = ND_SCALE * C.T
            E[R, 32 * k + R] = ND_SCALE
        scal = np.broadcast_to(
            np.array([Ls, Ld], dtype=np.float32)[None, :], (128, 2)
        )
        # rotate h so this core's own half sits in j-tiles 0..31
        h_rot = h if half == 0 else np.concatenate([h[HALF:], h[:HALF]], axis=0)
        m = {
            "hfull": np.ascontiguousarray(h_rot),
            "waug": wbd,
            "emat": E,
            "scal": np.ascontiguousarray(scal),
        }
        if has_bias:
            m["biasr"] = biasr
        in_maps.append(m)
    return has_bias, in_maps


def _run(h, w, a_src, a_dst, bias, trace=False, **trace_kwargs):
    has_bias, in_maps = _make_in_maps(h, w, a_src, a_dst, bias)
    nc = _get_nc(has_bias)
    res = run_bass_kernel_spmd(
        nc, in_maps, core_ids=list(range(8)), trace=trace, **trace_kwargs
    )
    out = np.zeros((BS, NH * F), dtype=np.float32)
    for c in range(8):
        head, half = c // 2, c % 2
        out[half * HALF : (half + 1) * HALF, head * F : (head + 1) * F] = res.results[
            c
        ]["out"]
    return out, res


def kernel(h, w, a_src, a_dst, bias):
    out, _ = _run(h, w, a_src, a_dst, bias, trace=False)
    return out


# revision 49
# speedup vs baseline: 1.0336x; 1.0336x over previous
"""Multi-head graph-attention (GAT) kernel for Trainium2, 8 NeuronCores.

Reference computation (per head):
    h_prime = h @ w[head]                       # [8192, 64]
    s = h_prime @ a_src[head],  d = h_prime @ a_dst[head]
    attn = softmax_j(leaky_relu(s_i + d_j, 0.2))
    out  = attn @ h_prime + bias                # -> [8192, 4*64]

Low-rank reformulation (no O(n^2) work on device):
    W[i,j] = exp(lrelu(s_i + d_j)) = e^{s_i} e^{d_j} + K(s_i, d_j)
  where K(s,d) = exp(0.2(s+d)) - exp(s+d) for s+d < 0, else 0, is a bounded
  continuous function on the (s,d) rectangle covered by the data.  K is
  approximated by a low-rank product expansion fitted ON THE HOST per head
  (ridge least squares; coefficients and ranges become runtime inputs):
    K(s,d) ~= sum_{a,b} C[a,b] f_a(s/Ls) g_b(d/Ld)
  f_a = Chebyshev T_a, a < S_RANK (serial DVE recurrence, small i side),
  g_b = tanh soft-steps {1, tanh(BETA(x - t_b))}, ND_RANK of them
  (independent, chain-free Tanh activations on the scalar engine for the
  large j side, issued in two row-halves overlapping the h' build; unused
  basis columns of the fixed 32-column layout are zeroed).
  With Haug = [h' | s^ d^ | 1] and TDaug = [g_b(d^) | e^d] (j on partitions):
    B      = TDaug^T @ Haug                      # [32, 67]   (PE, O(n R))
    BKaug4 = E^T @ B                             # [128, 67]  one matmul; E is
             host-built: rows 32k+a = sum_b C[a,b] B_gb + e^d row at 32k+31,
             replicated for k=0..3 and pre-scaled by 2^-8
    out^T  = Faug-tile^T @ BKaug4[32k:32k+32]    # [128 i, 67] per i-tile
  where Faug = [T_0..T_30 | e^s], transposed 4 i-tiles at a time (partition
  bands 0/32/64/96 + matching tile_position).  Row 66 of out^T is the softmax
  denominator; the epilogue divides (+bias) and stores.  Max rel err vs the
  reference is ~2.7e-3 (fp16 operands, fp32 psum accumulation): 7x inside
  the 2e-2 gate.  h' is computed with a host-built block-diagonal waug so
  one full-K matmul serves the two j-tiles stacked per hT2 block.

Sharding: 8 cores = 4 heads x 2 row-halves (head parallel + bs row shard).
Each core gets full h (rotated so its own half is first) plus per-head
host-fitted constants; no collectives.
"""

import numpy as np

import concourse.bass as bass
import concourse.tile as tile
from concourse import bacc, mybir
from concourse.bass_utils import run_bass_kernel_spmd
from concourse.masks import make_identity

F32 = mybir.dt.float32
F16 = mybir.dt.float16
AF = mybir.ActivationFunctionType

BS = 8192          # nodes
F = 64             # f_in == f_out
NH = 4             # heads
HALF = BS // 2     # rows per core (row-half)
NT = BS // 128     # 64 j tiles
NTI = HALF // 128  # 32 i tiles
R = 31             # expansion rank per side
MB = R + 1         # TDaug / Faug columns (basis + e^x)
MH = 67            # Haug columns: h'(64), 2 zero, ones
S_RANK = 24        # Chebyshev rank actually used on the s side
ND_RANK = 10       # number of tanh soft-steps on the d side
BETA = 5.0         # tanh soft-step sharpness (d-side basis)
SHIFT_EXT = 1.1    # shift extent of the tanh steps
ALPHA = 0.2
ND_SCALE = 2.0 ** -8   # folded into E so num/den stay small


def _build_kernel_module(has_bias):
    nc = bacc.Bacc("TRN2", target_bir_lowering=False, debug=False)

    h_d = nc.dram_tensor("hfull", [BS, F], F32, kind="ExternalInput")
    # waug_bd: block-diagonal [waug 0; 0 waug], waug = [w | ws/Ls | wd/Ld]
    waug_d = nc.dram_tensor("waug", [128, 2 * (F + 2)], F32,
                            kind="ExternalInput")
    # E: [MB, 128] expansion matrix (fit coeffs -> 4x replicated BKaug rows)
    e_d = nc.dram_tensor("emat", [MB, 128], F32, kind="ExternalInput")
    # scal: [128, 2] (Ls, Ld) replicated across partitions
    scal_d = nc.dram_tensor("scal", [128, 2], F32, kind="ExternalInput")
    if has_bias:
        biasr_d = nc.dram_tensor("biasr", [128, F], F32, kind="ExternalInput")
    out_d = nc.dram_tensor("out", [HALF, F], F32, kind="ExternalOutput")

    with tile.TileContext(nc) as tc:
        with (
            tc.tile_pool(name="const", bufs=1) as cpool,
            tc.tile_pool(name="work", bufs=3) as wpool,
            tc.tile_pool(name="psum", bufs=2, space="PSUM") as ppool,
        ):
            # first h block DMA goes on the queue BEFORE the const DMAs so
            # the PE can start transposing as early as possible
            hview = h_d.ap().rearrange("(a p) f -> p a f", p=128)
            ldb0 = wpool.tile([128, 8 * F], F32, tag="ldb", bufs=4)
            nc.sync.dma_start(ldb0[:], hview[:, 0:8, :])

            # ---------------- constants ----------------
            ident32 = cpool.tile([128, 128], F32)
            make_identity(nc, ident32[:])
            ident16 = cpool.tile([128, 128], F16)
            make_identity(nc, ident16[:])

            # block-diagonal waug: one full-K matmul emits h' for the two
            # j-tiles stacked on the partition halves of an hT2 block
            waug_sb = cpool.tile([128, 2 * (F + 2)], F32)
            nc.sync.dma_start(waug_sb[:], waug_d.ap())
            waug16 = cpool.tile([128, 2 * (F + 2)], F16)
            nc.vector.tensor_copy(waug16[:], waug_sb[:])

            e_sb = cpool.tile([MB, 128], F32)
            nc.sync.dma_start(e_sb[:], e_d.ap())
            e16 = cpool.tile([MB, 128], F16)

            scal_sb = cpool.tile([128, 2], F32)
            nc.sync.dma_start(scal_sb[:], scal_d.ap())
            if has_bias:
                biasr = cpool.tile([128, F], F32)
                nc.sync.dma_start(biasr[:], biasr_d.ap())

            # ---------------- big SBUF tensors ----------------
            # h^T in 2-tile blocks: block t holds j-tiles (2t, 2t+1) on
            # partition halves 0-63 / 64-127
            hT2 = cpool.tile([128, (NT // 2) * 128], F16)
            Haug = cpool.tile([128, NT * MH], F16)      # [h' | 0 0 | 1]
            Haug3 = Haug[:].rearrange("p (t c) -> p t c", c=MH)
            sd = cpool.tile([128, NT * 2], F32)         # [s^, d^] per j tile
            sd3 = sd[:].rearrange("p (t c) -> p t c", c=2)
            TDaug = cpool.tile([128, NT * MB], F16)     # [tanh-steps(d^) | e^d]
            TDaug3 = TDaug[:].rearrange("p (t c) -> p t c", c=MB)
            Fi = cpool.tile([128, NTI * MB], F16)       # [cheb(s^) | e^s]
            Fi3 = Fi[:].rearrange("p (t c) -> p t c", c=MB)
            S_all = cpool.tile([128, R * NTI], F32)     # cheb fp32 state
            S3 = S_all[:].rearrange("p (b t) -> p b t", t=NTI)
            # Faug^T: block q holds i-tiles 4q..4q+3 on partition bands 32k
            F3t = cpool.tile([128, (NTI // 4) * 128], F16)
            F33 = F3t[:].rearrange("p (t c) -> p t c", c=128)
            BKaug4 = cpool.tile([128, MH], F16)

            def init_memsets():
                nc.gpsimd.memset(Haug3[:, :, F : F + 2], 0.0)
                nc.gpsimd.memset(Haug3[:, :, MH - 1], 1.0)
                nc.gpsimd.memset(TDaug3[:, :, 0], 1.0)
                nc.gpsimd.memset(TDaug3[:, :, 1 + ND_RANK : MB - 1], 0.0)
                nc.gpsimd.memset(Fi3[:, :, S_RANK : MB - 1], 0.0)
                nc.gpsimd.memset(Fi3[:, :, 0], 1.0)
                nc.gpsimd.memset(S3[:, 0, :], 1.0)

            def cp(e, out, in_):
                (e.copy if e is nc.scalar else e.tensor_copy)(out, in_)

            # ---------------- phase 1 block worker ----------------
            # PSUM-reading copies only on vector/scalar (GPSIMD has no PSUM
            # access).  eng_ht drains the transposes, eng_hs drains h'.
            def phase1_block(blk, eng_ht, eng_hs, ldb=None, f32tr=False,
                             merge_sd=False):
                if ldb is None:
                    ldb = wpool.tile([128, 8 * F], F32, tag="ldb", bufs=4)
                    nc.sync.dma_start(
                        ldb[:], hview[:, blk * 8 : (blk + 1) * 8, :]
                    )
                if f32tr:
                    ldb16 = ldb
                    ident, tdt = ident32, F32
                else:
                    ldb16 = wpool.tile([128, 8 * F], F16, tag="ldb16", bufs=4)
                    nc.gpsimd.tensor_copy(ldb16[:], ldb[:])
                    ident, tdt = ident16, F16
                for g in range(2):
                    jt0 = blk * 8 + g * 4
                    trp2 = ppool.tile([128, 256], tdt, tag="tr", bufs=3)
                    for k in range(2):
                        nc.tensor.transpose(
                            trp2[:, k * 128 : (k + 1) * 128],
                            ldb16[:, (g * 4 + 2 * k) * F : (g * 4 + 2 * k + 2) * F],
                            ident[:],
                        )
                    blk2 = jt0 // 2
                    cp(eng_ht[g], hT2[:, blk2 * 128 : (blk2 + 2) * 128], trp2[:])
                    hp4 = ppool.tile([128, 4 * (F + 2)], F32, tag="hp", bufs=4)
                    for b2 in range(2):
                        nc.tensor.matmul(
                            hp4[:, b2 * 2 * (F + 2) : (b2 + 1) * 2 * (F + 2)],
                            hT2[:, (blk2 + b2) * 128 : (blk2 + b2 + 1) * 128],
                            waug16[:],
                        )
                    h3 = hp4[:].rearrange("p (t c) -> p t c", c=F + 2)
                    if merge_sd:
                        cp(eng_hs[g], Haug3[:, jt0 : jt0 + 4, 0 : F + 2],
                           h3[:, :, 0 : F + 2])
                    else:
                        cp(eng_hs[g], Haug3[:, jt0 : jt0 + 4, 0:F],
                           h3[:, :, 0:F])
                        cp(eng_hs[(g + 1) % 2], sd3[:, jt0 : jt0 + 4, :],
                           h3[:, :, F : F + 2])

            # ---------------- d side: tanh soft-step basis via scalar ACTs --
            # g_b(x) = tanh(BETA * (x - t_b)): one chain-free ACT per column
            shifts = np.linspace(-SHIFT_EXT, SHIFT_EXT, ND_RANK)
            tb_bias = cpool.tile([128, ND_RANK], F32)

            def d_tanh(lo, hi, b0=0, b1=ND_RANK, with_exp=True, from_haug=False):
                dv = Haug3[:, lo:hi, F + 1] if from_haug else sd3[:, lo:hi, 1]
                if with_exp:
                    nc.scalar.activation(
                        TDaug3[:, lo:hi, MB - 1], dv, AF.Exp,
                        scale=scal_sb[:, 1:2],
                    )
                for b in range(b0, b1):
                    nc.scalar.activation(
                        TDaug3[:, lo:hi, 1 + b], dv, AF.Tanh,
                        scale=BETA, bias=tb_bias[:, b : b + 1],
                    )

            # ---------------- s side: Chebyshev recurrence on DVE ----------
            s_view = sd3[:, 0:NTI, 0]
            x2s = cpool.tile([128, NTI], F32)

            def s_cheb_setup():
                nc.scalar.activation(
                    Fi3[:, :, MB - 1], s_view, AF.Exp, scale=scal_sb[:, 0:1]
                )
                nc.gpsimd.tensor_copy(Fi3[:, :, 1], s_view)
                nc.vector.tensor_copy(S3[:, 1, :], s_view)
                nc.vector.tensor_scalar_mul(x2s[:], s_view, 2.0)

            def s_cheb_chunk(b0, b1):
                for b in range(b0, b1):
                    tmp = wpool.tile([128, NTI], F32, tag="tmps", bufs=2)
                    nc.vector.tensor_mul(tmp[:], x2s[:], S3[:, b - 1, :])
                    nc.vector.tensor_sub(S3[:, b, :], tmp[:], S3[:, b - 2, :])
                    if b % 2 == 0:
                        nc.gpsimd.tensor_copy(
                            Fi3[:, :, b - 1 : b + 1],
                            S3[:, b - 1 : b + 1, :].rearrange("p b t -> p t b"),
                        )
                    elif b == S_RANK - 1:
                        nc.gpsimd.tensor_copy(
                            Fi3[:, :, b : b + 1],
                            S3[:, b : b + 1, :].rearrange("p b t -> p t b"),
                        )

            # ---------------- emit program ----------------
            phase1_block(0, [nc.vector, nc.scalar], [nc.scalar, nc.vector],
                         ldb=ldb0, f32tr=True)
            phase1_block(1, [nc.scalar, nc.vector], [nc.vector, nc.scalar])
            init_memsets()
            nc.gpsimd.tensor_copy(e16[:], e_sb[:])
            for b, t in enumerate(shifts):
                nc.gpsimd.memset(tb_bias[:, b : b + 1], float(-BETA * t))
            for blk in range(2, 4):
                et = [nc.vector, nc.scalar] if blk % 2 else [nc.scalar, nc.vector]
                phase1_block(blk, et, et[::-1])
            s_cheb_setup()
            d_tanh(0, 32)               # scalar, overlaps blocks 4-7
            s_chunks = [(2, 8), (8, 13), (13, 18), (18, S_RANK)]
            for blk in range(4, 8):
                phase1_block(blk, [nc.vector, nc.vector], [nc.scalar, nc.scalar],
                             merge_sd=True)
                s_cheb_chunk(*s_chunks[blk - 4])
            d_tanh(32, 64, from_haug=True)  # after block 7's merged drain

            # ---------------- B = TDaug^T @ Haug ----------------
            # split in halves so the Faug transposes can fill the PE while
            # the second-half d basis finishes
            B_ps = ppool.tile([MB, MH], F32, tag="acc", bufs=1)
            for jt in range(32):
                nc.tensor.matmul(
                    B_ps[:], TDaug3[:, jt, :], Haug3[:, jt, :],
                    start=(jt == 0), stop=False,
                )

            # transpose Faug (4 i-tiles per go)
            for q in range(NTI // 4):
                ftp = ppool.tile([128, 128], F16, tag="tr", bufs=3)
                nc.tensor.transpose(
                    ftp[:], Fi3[:, 4 * q : 4 * q + 4, :], ident16[:]
                )
                cp((nc.vector, nc.scalar)[q % 2], F33[:, q, :], ftp[:])

            for jt in range(32, NT):
                nc.tensor.matmul(
                    B_ps[:], TDaug3[:, jt, :], Haug3[:, jt, :],
                    start=False, stop=(jt == NT - 1),
                )
            B16 = cpool.tile([MB, MH], F16)
            nc.vector.tensor_copy(B16[:], B_ps[:])

            # BKaug4 = E^T @ B  (4x replicated, pre-scaled)
            bk_ps = ppool.tile([128, MH], F32, tag="tr", bufs=3)
            nc.tensor.matmul(bk_ps[:], e16[:], B16[:])
            nc.vector.tensor_copy(BKaug4[:], bk_ps[:])

            # ---------------- synthesis + epilogue ----------------
            out_view = out_d.ap().rearrange("(a p) f -> p a f", p=128)
            for ch in range(NTI // 4):
                o1c = wpool.tile([128, 4 * F], F32, tag="o1c", bufs=3)
                for sub in range(4):
                    it = ch * 4 + sub
                    q, k = it // 4, it % 4
                    lo = 32 * k
                    ot_ps = ppool.tile([128, MH], F32, tag="hp", bufs=4)
                    nc.tensor.matmul(
                        ot_ps[:],
                        F33[lo : lo + 32, q, :],
                        BKaug4[lo : lo + 32, :],
                        tile_position=(lo, 0),
                    )
                    rec = wpool.tile([128, 1], F32, tag="rec", bufs=4)
                    nc.vector.reciprocal(rec[:], ot_ps[:, MH - 1 : MH])
                    if has_bias:
                        o1 = wpool.tile([128, F], F32, tag="o1", bufs=4)[:]
                    else:
                        o1 = o1c[:, sub * F : (sub + 1) * F]
                    if sub % 2 == 0:
                        nc.scalar.mul(o1, ot_ps[:, 0:F], rec[:])
                    else:
                        nc.vector.tensor_scalar_mul(o1, ot_ps[:, 0:F], rec[:])
                    if has_bias:
                        nc.gpsimd.tensor_add(
                            o1c[:, sub * F : (sub + 1) * F], o1, biasr[:]
                        )
                nc.sync.dma_start(
                    out_view[:, ch * 4 : (ch + 1) * 4, :], o1c[:]
                )

    nc.compile()
    return nc


_NC_CACHE = {}


def _get_nc(has_bias):
    if has_bias not in _NC_CACHE:
        _NC_CACHE[has_bias] = _build_kernel_module(has_bias)
    return _NC_CACHE[has_bias]


def _step_basis(xh):
    cols = [np.ones_like(xh)]
    for t in np.linspace(-SHIFT_EXT, SHIFT_EXT, ND_RANK):
        cols.append(np.tanh(BETA * (xh - t)))
    return np.stack(cols, axis=1)


def _fit_K(Ls, Ld, ngrid=160, lam=1e-10):
    """Least-squares fit of K(s,d) = exp(.2(s+d))-exp(s+d) (s+d<0) over
    [-Ls,Ls] x [-Ld,Ld] in the product basis cheb(s) x fourier(d).
    Returns C [R, R] (s-basis x d-basis)."""
    gs = np.linspace(-1.0, 1.0, ngrid)
    S, D = np.meshgrid(gs * Ls, gs * Ld, indexing="ij")
    X = S + D
    K = np.where(X < 0, np.exp(ALPHA * X) - np.exp(X), 0.0)
    Bs = np.polynomial.chebyshev.chebvander(gs, S_RANK - 1)
    Bd = _step_basis(gs)

    def pinv(B):
        U, sv, Vt = np.linalg.svd(B, full_matrices=False)
        return (Vt.T * (sv / (sv ** 2 + lam))) @ U.T

    return pinv(Bs) @ K @ pinv(Bd).T


def _make_in_maps(h, w, a_src, a_dst, bias):
    h = np.ascontiguousarray(np.asarray(h, dtype=np.float32))
    w = np.asarray(w, dtype=np.float32)
    a_src = np.asarray(a_src, dtype=np.float32)
    a_dst = np.asarray(a_dst, dtype=np.float32)
    bias = np.asarray(bias, dtype=np.float32)
    has_bias = bool(np.any(bias != 0.0))
    biasr = np.ascontiguousarray(np.broadcast_to(bias[None, :], (128, F)))

    in_maps = []
    for c in range(8):
        head, half = c // 2, c % 2
        ws = w[head] @ a_src[head][:, 0]          # [64]
        wd = w[head] @ a_dst[head][:, 0]
        s_all = h @ ws
        d_all = h @ wd
        Ls = float(np.abs(s_all).max()) * 1.02 + 1e-30
        Ld = float(np.abs(d_all).max()) * 1.02 + 1e-30
        C = _fit_K(Ls, Ld)
        waug = np.concatenate(
            [w[head], (ws / Ls)[:, None], (wd / Ld)[:, None]], axis=1
        ).astype(np.float32)
        wbd = np.zeros((128, 2 * (F + 2)), dtype=np.float32)
        wbd[0:F, 0 : F + 2] = waug
        wbd[F:128, F + 2 : 2 * (F + 2)] = waug
        # E[b, 32k+a] = scale*C[a, b]; unused rows/cols stay 0; e^d at row R
        E = np.zeros((MB, 128), dtype=np.float32)
        for k in range(4):
            E[0 : 1 + ND_RANK, 32 * k : 32 * k + S_RANK] = ND_SCALE * C.T
            E[R, 32 * k + R] = ND_SCALE
        scal = np.broadcast_to(
            np.array([Ls, Ld], dtype=np.float32)[None, :], (128, 2)
        )
        # rotate h so this core's own half sits in j-tiles 0..31
        h_rot = h if half == 0 else np.concatenate([h[HALF:], h[:HALF]], axis=0)
        m = {
            "hfull": np.ascontiguousarray(h_rot),
            "waug": wbd,
            "emat": E,
            "scal": np.ascontiguousarray(scal),
        }
        if has_bias:
            m["biasr"] = biasr
        in_maps.append(m)
    return has_bias, in_maps


def _run(h, w, a_src, a_dst, bias, trace=False, **trace_kwargs):
    has_bias, in_maps = _make_in_maps(h, w, a_src, a_dst, bias)
    nc = _get_nc(has_bias)
    res = run_bass_kernel_spmd(
        nc, in_maps, core_ids=list(range(8)), trace=trace, **trace_kwargs
    )
    out = np.zeros((BS, NH * F), dtype=np.float32)
    for c in range(8):
        head, half = c // 2, c % 2
        out[half * HALF : (half + 1) * HALF, head * F : (head + 1) * F] = res.results[
            c
        ]["out"]
    return out, res


def kernel(h, w, a_src, a_dst, bias):
    out, _ = _run(h, w, a_src, a_dst, bias, trace=False)
    return out
